# revision 1
# baseline (speedup 1.0000x reference)
"""Trainium2 Bass kernel for nn_M02SameVQ (VQ codebook match + refiner MLP).

Strategy (8 NeuronCores, data-parallel over batch):
 - Coarse nearest-centroid scores s = x.c - 0.5||c||^2 computed in fp16 on
   the PE (full rate), bias folded in as a K=2 matmul of split-fp16 rows.
 - Top-8 scan on VectorE (fp16), top-4 candidates re-scored exactly in fp32
   (dma_gather of extended codebook rows + gpsimd multiply + DVE reduce),
   running-argmax select reproduces the fp32 reference argmin exactly.
 - lin_dec selected from gathered rows, PE-transposed back to channel-major.
 - Refiner MLP runs in fp16 (spk_dec is ~0.3% of output magnitude; fp16
   error is ~1e-5 relative end-to-end). Output assembled in fp32.
"""

import numpy as np
import concourse.tile as tile
import concourse.mybir as mybir
from concourse import bacc, bass_utils
from concourse.masks import make_identity

F32 = mybir.dt.float32
F16 = mybir.dt.float16
I16 = mybir.dt.int16
U32 = mybir.dt.uint32
U8 = mybir.dt.uint8
AF = mybir.ActivationFunctionType
ALU = mybir.AluOpType
AX = mybir.AxisListType

B, C, T = 16, 1024, 1500
NBINS, HID, EMB = 4096, 512, 256
NCORES = 8
BPC = B // NCORES          # batches per core
NT = 125                   # tokens per distance tile
TPB = T // NT              # 12 tiles per batch
BLK = 250                  # tokens per MLP block (2 tiles)
TBLK = BLK // NT
NBLK = T // BLK            # 6 blocks per batch
CE = 1088                  # ext codebook row: 1024 cen | -0.5||c||^2 | 1.0 | pad
KR = 4                     # rescue candidates
KG = C // 128              # 8 contraction chunks
NJ = NBINS // 512          # 8 bin-tiles

_CACHE = {}


def _build_body(nc, tc, d):
    with tc.tile_pool(name="const", bufs=1) as cp, \
         tc.tile_pool(name="feat", bufs=1) as fp, \
         tc.tile_pool(name="work", bufs=1) as wp, \
         tc.tile_pool(name="mlp", bufs=1) as mp, \
         tc.tile_pool(name="ps", bufs=1, space="PSUM") as ps, \
         tc.tile_pool(name="dr", bufs=1, space="DRAM") as dr:

        # ---- resident constants ----
        cs16 = cp.tile([128, KG, NBINS], F16)
        nc.sync.dma_start(out=cs16, in_=d["cT16"].rearrange("(g p) b -> p g b", p=128))
        bias2 = cp.tile([2, NBINS], F16)
        nc.sync.dma_start(out=bias2, in_=d["bias2"])
        ones2 = cp.tile([2, 128], F16)
        nc.vector.memset(ones2, 1.0)
        ident = cp.tile([128, 128], F32)
        make_identity(nc, ident)

        w0 = cp.tile([128, KG, HID], F16)
        nc.sync.dma_start(out=w0, in_=d["w0T"].rearrange("(g p) h -> p g h", p=128))
        w1 = cp.tile([128, 4, HID], F16)
        nc.sync.dma_start(out=w1, in_=d["w1T"].rearrange("(g p) h -> p g h", p=128))
        w2 = cp.tile([128, 4, EMB], F16)
        nc.sync.dma_start(out=w2, in_=d["w2T"].rearrange("(g p) h -> p g h", p=128))
        v0 = cp.tile([128, 2, HID], F16)
        nc.sync.dma_start(out=v0, in_=d["v0T"].rearrange("(g p) h -> p g h", p=128))
        v1 = cp.tile([128, 4, HID], F16)
        nc.sync.dma_start(out=v1, in_=d["v1T"].rearrange("(g p) h -> p g h", p=128))
        v2 = cp.tile([128, 4, C], F16)
        nc.sync.dma_start(out=v2, in_=d["v2T"].rearrange("(g p) h -> p g h", p=128))
        b0 = cp.tile([128, 4], F32)
        nc.sync.dma_start(out=b0, in_=d["b0"].rearrange("(g p) -> p g", p=128))
        b1 = cp.tile([128, 4], F32)
        nc.sync.dma_start(out=b1, in_=d["b1"].rearrange("(g p) -> p g", p=128))
        b2 = cp.tile([128, 2], F32)
        nc.sync.dma_start(out=b2, in_=d["b2"].rearrange("(g p) -> p g", p=128))
        c0 = cp.tile([128, 4], F32)
        nc.sync.dma_start(out=c0, in_=d["c0"].rearrange("(g p) -> p g", p=128))
        c1 = cp.tile([128, 4], F32)
        nc.sync.dma_start(out=c1, in_=d["c1"].rearrange("(g p) -> p g", p=128))
        c2 = cp.tile([128, KG], F32)
        nc.sync.dma_start(out=c2, in_=d["c2"].rearrange("(g p) -> p g", p=128))

        for batch in range(BPC):
            feat_b = d["feat"][batch].rearrange("(g p) t -> p g t", p=128)
            out_b = d["out"][batch].rearrange("(g p) t -> p g t", p=128)
            for blk in range(NBLK):
                tb0 = blk * BLK
                fs32 = fp.tile([128, KG, BLK], F32, tag="fs32")
                nc.sync.dma_start(out=fs32, in_=feat_b[:, :, tb0:tb0 + BLK])
                fs16 = fp.tile([128, KG, BLK], F16, tag="fs16")
                nc.scalar.activation(fs16, fs32, AF.Copy)
                linT = fp.tile([128, KG, BLK], F32, tag="linT")
                spk16 = fp.tile([128, KG, BLK], F16, tag="spk16")

                for t in range(TBLK):
                    tsl = slice(t * NT, (t + 1) * NT)
                    # ---- coarse fp16 scores ----
                    dist16 = wp.tile([128, NBINS], F16, tag="dist16")
                    for j in range(NJ):
                        pj = ps.tile([NT, 512], F32, tag="dist")
                        for g in range(KG):
                            nc.tensor.matmul(pj, lhsT=fs16[:, g, tsl],
                                             rhs=cs16[:, g, j * 512:(j + 1) * 512],
                                             start=(g == 0), stop=False)
                        nc.tensor.matmul(pj, lhsT=ones2[:, :NT],
                                         rhs=bias2[:, j * 512:(j + 1) * 512],
                                         start=False, stop=True)
                        nc.scalar.activation(dist16[:NT, j * 512:(j + 1) * 512],
                                             pj, AF.Copy)
                    # ---- top-8 scan ----
                    max8 = wp.tile([NT, 8], F16, tag="max8")
                    idx8 = wp.tile([NT, 8], U32, tag="idx8")
                    nc.vector.max(out=max8, in_=dist16[:NT])
                    nc.vector.max_index(out=idx8, in_max=max8, in_values=dist16[:NT])
                    # ---- wrapped int16 idx lists for KR candidates ----
                    idx16c = wp.tile([128, KR], I16, tag="idx16c")
                    nc.vector.memset(idx16c, 0)
                    nc.vector.tensor_copy(out=idx16c[:NT], in_=idx8[:, :KR])
                    scr = dr.tile([128, KR], I16, tag="scr")
                    nc.sync.dma_start(out=scr, in_=idx16c)
                    idxw = wp.tile([128, KR * 8], I16, tag="idxw")
                    for k in range(KR):
                        wsrc = scr[:, k].rearrange("(j p) -> p j", p=16)
                        for r in range(8):
                            nc.sync.dma_start(
                                out=idxw[r * 16:(r + 1) * 16, k * 8:(k + 1) * 8],
                                in_=wsrc)
                    # ---- gathers + exact rescue ----
                    gs = []
                    for k in range(KR):
                        gk = wp.tile([128, 1, CE], F32, tag="gath")
                        nc.gpsimd.dma_gather(
                            out_ap=gk, in_ap=d["cen_ext"],
                            idxs_ap=idxw[:, k * 8:(k + 1) * 8],
                            num_idxs=128, num_idxs_reg=128, elem_size=CE)
                        gs.append(gk)
                    x_t = wp.tile([NT, CE], F32, tag="x_t")
                    for g in range(KG):
                        tp = ps.tile([128, 128], F32, tag="tpose")
                        nc.tensor.transpose(tp[:NT], fs32[:, g, tsl], ident)
                        nc.scalar.activation(x_t[:, g * 128:(g + 1) * 128],
                                             tp[:NT], AF.Copy)
                    nc.vector.memset(x_t[:, C:C + 1], 1.0)
                    sex = []
                    for k in range(KR):
                        prod = wp.tile([NT, C + 1], F32, tag="prod")
                        nc.gpsimd.tensor_mul(prod, x_t[:, :C + 1],
                                             gs[k][:NT, 0, :C + 1])
                        sk = wp.tile([NT, 1], F32, tag=f"sex{k}")
                        nc.vector.tensor_reduce(sk, prod, AX.X, ALU.add)
                        sex.append(sk)
                    # ---- running argmax over exact scores ----
                    bv = wp.tile([NT, 1], F32, tag="bv")
                    nc.vector.tensor_copy(bv, sex[0])
                    lin_sel = wp.tile([NT, C], F32, tag="lin_sel")
                    nc.vector.tensor_copy(lin_sel, gs[0][:NT, 0, :C])
                    for k in range(1, KR):
                        mk = wp.tile([NT, 1], U8, tag=f"mk{k}")
                        nc.vector.tensor_tensor(mk, sex[k], bv, ALU.is_gt)
                        nc.vector.copy_predicated(bv, mk, sex[k])
                        nc.vector.copy_predicated(
                            lin_sel, mk.to_broadcast([NT, C]), gs[k][:NT, 0, :C])
                    # ---- transpose lin_sel back to C-major (+db2 fold) ----
                    for g in range(KG):
                        tp2 = ps.tile([128, 128], F32, tag="tpose")
                        nc.tensor.transpose(tp2[:, :NT],
                                            lin_sel[:, g * 128:(g + 1) * 128],
                                            ident[:NT, :NT])
                        nc.scalar.activation(linT[:, g, tsl], tp2[:, :NT],
                                             AF.Identity, bias=c2[:, g:g + 1])
                    # ---- speaker residual (fp16 MLP input) ----
                    nc.gpsimd.tensor_sub(spk16[:, :, tsl], fs32[:, :, tsl],
                                         linT[:, :, tsl])

                # ---- refiner MLP on the 250-token block ----
                h1 = mp.tile([128, 4, BLK], F16, tag="h1")
                for m in range(4):
                    pm = ps.tile([128, BLK], F32, tag="mlp")
                    for g in range(KG):
                        nc.tensor.matmul(pm, lhsT=w0[:, g, m * 128:(m + 1) * 128],
                                         rhs=spk16[:, g, :],
                                         start=(g == 0), stop=(g == KG - 1))
                    nc.scalar.activation(h1[:, m, :], pm, AF.Lrelu,
                                         bias=b0[:, m:m + 1], alpha=0.01)
                h2 = mp.tile([128, 4, BLK], F16, tag="h2")
                for m in range(4):
                    pm = ps.tile([128, BLK], F32, tag="mlp")
                    for g in range(4):
                        nc.tensor.matmul(pm, lhsT=w1[:, g, m * 128:(m + 1) * 128],
                                         rhs=h1[:, g, :],
                                         start=(g == 0), stop=(g == 3))
                    nc.scalar.activation(h2[:, m, :], pm, AF.Lrelu,
                                         bias=b1[:, m:m + 1], alpha=0.01)
                z = mp.tile([128, 2, BLK], F16, tag="z")
                for m in range(2):
                    pm = ps.tile([128, BLK], F32, tag="mlp")
                    for g in range(4):
                        nc.tensor.matmul(pm, lhsT=w2[:, g, m * 128:(m + 1) * 128],
                                         rhs=h2[:, g, :],
                                         start=(g == 0), stop=(g == 3))
                    nc.scalar.activation(z[:, m, :], pm, AF.Identity,
                                         bias=b2[:, m:m + 1])
                d1 = mp.tile([128, 4, BLK], F16, tag="d1")
                for m in range(4):
                    pm = ps.tile([128, BLK], F32, tag="mlp")
                    for g in range(2):
                        nc.tensor.matmul(pm, lhsT=v0[:, g, m * 128:(m + 1) * 128],
                                         rhs=z[:, g, :],
                                         start=(g == 0), stop=(g == 1))
                    nc.scalar.activation(d1[:, m, :], pm, AF.Lrelu,
                                         bias=c0[:, m:m + 1], alpha=0.01)
                d2 = mp.tile([128, 4, BLK], F16, tag="d2")
                for m in range(4):
                    pm = ps.tile([128, BLK], F32, tag="mlp")
                    for g in range(4):
                        nc.tensor.matmul(pm, lhsT=v1[:, g, m * 128:(m + 1) * 128],
                                         rhs=d1[:, g, :],
                                         start=(g == 0), stop=(g == 3))
                    nc.scalar.activation(d2[:, m, :], pm, AF.Lrelu,
                                         bias=c1[:, m:m + 1], alpha=0.01)
                out_sb = fp.tile([128, KG, BLK], F32, tag="out_sb")
                for m in range(KG):
                    pm = ps.tile([128, BLK], F32, tag="mlp")
                    for g in range(4):
                        nc.tensor.matmul(pm, lhsT=v2[:, g, m * 128:(m + 1) * 128],
                                         rhs=d2[:, g, :],
                                         start=(g == 0), stop=(g == 3))
                    nc.vector.tensor_tensor(out_sb[:, m, :], pm, linT[:, m, :],
                                            ALU.add)
                nc.sync.dma_start(out=out_b[:, :, tb0:tb0 + BLK], in_=out_sb)


def build_nc():
    nc = bacc.Bacc("TRN2", target_bir_lowering=False, debug=False,
                   enable_asserts=False, num_devices=NCORES)
    d = {}
    d["feat"] = nc.dram_tensor("feat", (BPC, C, T), F32, kind="ExternalInput").ap()
    d["cT16"] = nc.dram_tensor("cT16", (C, NBINS), F16, kind="ExternalInput").ap()
    d["bias2"] = nc.dram_tensor("bias2", (2, NBINS), F16, kind="ExternalInput").ap()
    d["cen_ext"] = nc.dram_tensor("cen_ext", (NBINS, CE), F32,
                                  kind="ExternalInput").ap()
    for nm, shp in [("w0T", (C, HID)), ("w1T", (HID, HID)), ("w2T", (HID, EMB)),
                    ("v0T", (EMB, HID)), ("v1T", (HID, HID)), ("v2T", (HID, C))]:
        d[nm] = nc.dram_tensor(nm, shp, F16, kind="ExternalInput").ap()
    for nm, n in [("b0", HID), ("b1", HID), ("b2", EMB),
                  ("c0", HID), ("c1", HID), ("c2", C)]:
        d[nm] = nc.dram_tensor(nm, (n,), F32, kind="ExternalInput").ap()
    d["out"] = nc.dram_tensor("out", (BPC, C, T), F32, kind="ExternalOutput").ap()

    with tile.TileContext(nc) as tc:
        _build_body(nc, tc, d)
    nc.compile()
    return nc


def _prep_shared(centroid, ew0, eb0, ew1, eb1, ew2, eb2, dw0, db0, dw1, db1,
                 dw2, db2):
    cen = np.asarray(centroid, np.float32)
    c_norm = (cen.astype(np.float64) ** 2).sum(1)
    bias32 = (-0.5 * c_norm).astype(np.float32)
    bias_c = bias32 - np.float32(bias32.mean())
    b_hi = bias_c.astype(np.float16)
    b_lo = (bias_c - b_hi.astype(np.float32)).astype(np.float16)
    cen_ext = np.zeros((NBINS, CE), np.float32)
    cen_ext[:, :C] = cen
    cen_ext[:, C] = bias32
    cen_ext[:, C + 1] = 1.0
    shared = {
        "cT16": np.ascontiguousarray(cen.T).astype(np.float16),
        "bias2": np.stack([b_hi, b_lo]),
        "cen_ext": cen_ext,
        "w0T": np.ascontiguousarray(np.asarray(ew0).T).astype(np.float16),
        "w1T": np.ascontiguousarray(np.asarray(ew1).T).astype(np.float16),
        "w2T": np.ascontiguousarray(np.asarray(ew2).T).astype(np.float16),
        "v0T": np.ascontiguousarray(np.asarray(dw0).T).astype(np.float16),
        "v1T": np.ascontiguousarray(np.asarray(dw1).T).astype(np.float16),
        "v2T": np.ascontiguousarray(np.asarray(dw2).T).astype(np.float16),
        "b0": np.asarray(eb0, np.float32), "b1": np.asarray(eb1, np.float32),
        "b2": np.asarray(eb2, np.float32), "c0": np.asarray(db0, np.float32),
        "c1": np.asarray(db1, np.float32), "c2": np.asarray(db2, np.float32),
    }
    return shared


def _get_nc():
    if "nc" not in _CACHE:
        _CACHE["nc"] = build_nc()
    return _CACHE["nc"]


def run(inputs, trace=False):
    feature = np.ascontiguousarray(np.asarray(inputs["feature"], np.float32))
    shared = _prep_shared(**{k: v for k, v in inputs.items() if k != "feature"})
    nc = _get_nc()
    in_maps = []
    for c in range(NCORES):
        m = dict(shared)
        m["feat"] = np.ascontiguousarray(feature[c * BPC:(c + 1) * BPC])
        in_maps.append(m)
    kw = {}
    if trace:
        kw = dict(trace=True, trace_cores=list(range(NCORES)))
    res = bass_utils.run_bass_kernel_spmd(nc, in_maps, core_ids=list(range(NCORES)),
                                          **kw)
    out = np.empty((B, C, T), np.float32)
    for c in range(NCORES):
        out[c * BPC:(c + 1) * BPC] = res.results[c]["out"]
    return out, res


def kernel(**inputs) -> np.ndarray:
    out, _ = run(inputs, trace=False)
    return out



# revision 7
# speedup vs baseline: 1.8927x; 1.8927x over previous
"""Trainium2 Bass kernel for nn_M02SameVQ (VQ codebook match + refiner MLP).

Strategy (8 NeuronCores, data-parallel over batch, 2 batches/core):
 - Tokens padded 1500->1536, processed in 128-token tiles (12/batch).
 - Coarse scores s = x.c - 0.5||c||^2 in fp16 on the PE; bias folded in as a
   K=2 matmul of split-fp16 rows (exact to ~1e-4).
 - Top-8 scan on VectorE (fp16); top-3 candidates re-scored exactly in fp32:
   one dma_gather of 3 extended codebook rows per token (vs a host
   pre-transposed fp32 token-major feature copy), gpsimd multiply + DVE
   reduce, running-argmax on the scores selects the winner INDEX only.
 - lin_dec arrives channel-major directly via a transposed fp16 dma_gather
   of the winner row (+db2 pre-folded on host). PE does no transposes.
 - Refiner MLP in fp16 over 256-token blocks, software-pipelined two blocks
   behind the match so the PE never waits on the rescue chain.
 - Wrapped gather-index lists built with a 2-DMA DRAM bounce (fancy-AP write
   into wrapped layout + single broadcast-AP read).
"""

import numpy as np
import concourse.tile as tile
import concourse.mybir as mybir
from concourse import bacc, bass_utils

F32 = mybir.dt.float32
F16 = mybir.dt.float16
I16 = mybir.dt.int16
U32 = mybir.dt.uint32
U8 = mybir.dt.uint8
AF = mybir.ActivationFunctionType
ALU = mybir.AluOpType
AX = mybir.AxisListType

B, C, T = 16, 1024, 1500
NBINS, HID, EMB = 4096, 512, 256
NCORES = 8
BPC = B // NCORES          # batches per core
TP = 1536                  # padded tokens
NT = 128                   # tokens per tile
BLK = 256                  # tokens per MLP block (2 tiles)
TBLK = BLK // NT
NBLK = TP // BLK           # 6 blocks per batch
CE = 1088                  # ext codebook row: 1024 cen+db2 | -0.5||c||^2 | 1.0 | pad
CXT = 1032                 # xT row: 1024 x | 1.0 | pad (32B aligned)
KR = 3                     # rescue candidates
KG = C // 128              # 8 contraction chunks
NJ = NBINS // 512          # 8 bin-tiles

_CACHE = {}


def _build_body(nc, tc, d):
    with tc.tile_pool(name="const", bufs=1) as cp, \
         tc.tile_pool(name="blk", bufs=3) as bp, \
         tc.tile_pool(name="tile", bufs=2) as tp, \
         tc.tile_pool(name="small", bufs=4) as sp, \
         tc.tile_pool(name="mlp", bufs=1) as mp, \
         tc.tile_pool(name="psd", bufs=4, space="PSUM") as psd, \
         tc.tile_pool(name="psm", bufs=2, space="PSUM") as psm, \
         tc.tile_pool(name="dr", bufs=4, space="DRAM") as dr:

        # ---- resident constants ----
        cs16 = cp.tile([128, KG, NBINS], F16)
        nc.sync.dma_start(out=cs16, in_=d["cT16"].rearrange("(g p) b -> p g b", p=128))
        bias2 = cp.tile([2, NBINS], F16)
        nc.sync.dma_start(out=bias2, in_=d["bias2"])
        ones2 = cp.tile([2, 128], F16)
        nc.vector.memset(ones2, 1.0)

        w0 = cp.tile([128, KG, HID], F16)
        nc.sync.dma_start(out=w0, in_=d["w0T"].rearrange("(g p) h -> p g h", p=128))
        w1 = cp.tile([128, 4, HID], F16)
        nc.sync.dma_start(out=w1, in_=d["w1T"].rearrange("(g p) h -> p g h", p=128))
        w2 = cp.tile([128, 4, EMB], F16)
        nc.sync.dma_start(out=w2, in_=d["w2T"].rearrange("(g p) h -> p g h", p=128))
        v0 = cp.tile([128, 2, HID], F16)
        nc.sync.dma_start(out=v0, in_=d["v0T"].rearrange("(g p) h -> p g h", p=128))
        v1 = cp.tile([128, 4, HID], F16)
        nc.sync.dma_start(out=v1, in_=d["v1T"].rearrange("(g p) h -> p g h", p=128))
        v2 = cp.tile([128, 4, C], F16)
        nc.sync.dma_start(out=v2, in_=d["v2T"].rearrange("(g p) h -> p g h", p=128))
        b0 = cp.tile([128, 4], F32)
        nc.sync.dma_start(out=b0, in_=d["b0"].rearrange("(g p) -> p g", p=128))
        b1 = cp.tile([128, 4], F32)
        nc.sync.dma_start(out=b1, in_=d["b1"].rearrange("(g p) -> p g", p=128))
        b2 = cp.tile([128, 2], F32)
        nc.sync.dma_start(out=b2, in_=d["b2"].rearrange("(g p) -> p g", p=128))
        c0 = cp.tile([128, 4], F32)
        nc.sync.dma_start(out=c0, in_=d["c0"].rearrange("(g p) -> p g", p=128))
        c1 = cp.tile([128, 4], F32)
        nc.sync.dma_start(out=c1, in_=d["c1"].rearrange("(g p) -> p g", p=128))

        def emit_mlp(prev):
            """Refiner MLP + output store for a finished block."""
            spk16, linT16, out_b, tb0 = prev
            h1 = mp.tile([128, 4, BLK], F16, tag="h1")
            for m in range(4):
                pm = psm.tile([128, BLK], F32, tag="mlp")
                for g in range(KG):
                    nc.tensor.matmul(pm, lhsT=w0[:, g, m * 128:(m + 1) * 128],
                                     rhs=spk16[:, g, :],
                                     start=(g == 0), stop=(g == KG - 1))
                nc.scalar.activation(h1[:, m, :], pm, AF.Lrelu,
                                     bias=b0[:, m:m + 1], alpha=0.01)
            h2 = mp.tile([128, 4, BLK], F16, tag="h2")
            for m in range(4):
                pm = psm.tile([128, BLK], F32, tag="mlp")
                for g in range(4):
                    nc.tensor.matmul(pm, lhsT=w1[:, g, m * 128:(m + 1) * 128],
                                     rhs=h1[:, g, :],
                                     start=(g == 0), stop=(g == 3))
                nc.scalar.activation(h2[:, m, :], pm, AF.Lrelu,
                                     bias=b1[:, m:m + 1], alpha=0.01)
            z = mp.tile([128, 2, BLK], F16, tag="z")
            for m in range(2):
                pm = psm.tile([128, BLK], F32, tag="mlp")
                for g in range(4):
                    nc.tensor.matmul(pm, lhsT=w2[:, g, m * 128:(m + 1) * 128],
                                     rhs=h2[:, g, :],
                                     start=(g == 0), stop=(g == 3))
                nc.scalar.activation(z[:, m, :], pm, AF.Identity,
                                     bias=b2[:, m:m + 1])
            d1 = mp.tile([128, 4, BLK], F16, tag="h1")
            for m in range(4):
                pm = psm.tile([128, BLK], F32, tag="mlp")
                for g in range(2):
                    nc.tensor.matmul(pm, lhsT=v0[:, g, m * 128:(m + 1) * 128],
                                     rhs=z[:, g, :],
                                     start=(g == 0), stop=(g == 1))
                nc.scalar.activation(d1[:, m, :], pm, AF.Lrelu,
                                     bias=c0[:, m:m + 1], alpha=0.01)
            d2 = mp.tile([128, 4, BLK], F16, tag="h2")
            for m in range(4):
                pm = psm.tile([128, BLK], F32, tag="mlp")
                for g in range(4):
                    nc.tensor.matmul(pm, lhsT=v1[:, g, m * 128:(m + 1) * 128],
                                     rhs=d1[:, g, :],
                                     start=(g == 0), stop=(g == 3))
                nc.scalar.activation(d2[:, m, :], pm, AF.Lrelu,
                                     bias=c1[:, m:m + 1], alpha=0.01)
            for m in range(KG):
                pm = psm.tile([128, BLK], F32, tag="mlp")
                for g in range(4):
                    nc.tensor.matmul(pm, lhsT=v2[:, g, m * 128:(m + 1) * 128],
                                     rhs=d2[:, g, :],
                                     start=(g == 0), stop=(g == 3))
                outm = sp.tile([128, BLK], F32, tag="outm")
                nc.vector.tensor_tensor(
                    outm.rearrange("p (u t) -> p u t", u=TBLK),
                    pm.rearrange("p (u t) -> p u t", u=TBLK),
                    linT16[:, :, m, :], ALU.add)
                nc.sync.dma_start(out=out_b[:, m, tb0:tb0 + BLK], in_=outm)

        pending = []
        for batch in range(BPC):
            xh_b = d["xh"][batch].rearrange("(g p) t -> p g t", p=128)
            xT_b = d["xT"][batch]
            out_b = d["out"][batch].rearrange("(g p) t -> p g t", p=128)
            for blk in range(NBLK):
                tb0 = blk * BLK
                xhb = bp.tile([128, KG, BLK], F16, tag="xhb")
                nc.sync.dma_start(out=xhb, in_=xh_b[:, :, tb0:tb0 + BLK])
                spk16 = bp.tile([128, KG, BLK], F16, tag="spk16")
                linT16 = bp.tile([128, TBLK, KG, NT], F16, tag="linT16")

                for t in range(TBLK):
                    tsl = slice(t * NT, (t + 1) * NT)
                    tok0 = tb0 + t * NT
                    # ---- token-major fp32 x (incl trailing 1.0 col) ----
                    x_t = tp.tile([128, C + 1], F32, tag="x_t")
                    nc.sync.dma_start(out=x_t, in_=xT_b[tok0:tok0 + NT, :C + 1])
                    # ---- coarse fp16 scores ----
                    dist16 = tp.tile([128, NBINS], F16, tag="dist16")
                    for j in range(NJ):
                        pj = psd.tile([NT, 512], F32, tag="dist")
                        nc.tensor.matmul(pj, lhsT=ones2[:, :NT],
                                         rhs=bias2[:, j * 512:(j + 1) * 512],
                                         start=True, stop=False)
                        for g in range(KG):
                            nc.tensor.matmul(pj, lhsT=xhb[:, g, tsl],
                                             rhs=cs16[:, g, j * 512:(j + 1) * 512],
                                             start=False, stop=(g == KG - 1))
                        nc.scalar.activation(dist16[:, j * 512:(j + 1) * 512],
                                             pj, AF.Copy)
                    # ---- top-8 scan ----
                    max8 = sp.tile([NT, 8], F16, tag="max8")
                    idx8 = sp.tile([NT, 8], U32, tag="idx8")
                    nc.vector.max(out=max8, in_=dist16)
                    nc.vector.max_index(out=idx8, in_max=max8, in_values=dist16)
                    idx16c = sp.tile([128, KR], I16, tag="idx16c")
                    nc.vector.tensor_copy(out=idx16c, in_=idx8[:, :KR])
                    # ---- wrapped idx list: fancy write + broadcast read ----
                    scr = dr.tile([16, KR * 8], I16, tag="scr")
                    nc.scalar.dma_start(out=scr.rearrange("q (k j) -> j q k", j=8),
                                        in_=idx16c)
                    idxw = sp.tile([128, KR * 8], I16, tag="idxw")
                    nc.scalar.dma_start(
                        out=idxw,
                        in_=scr.unsqueeze(0).broadcast_to([8, 16, KR * 8]))
                    # ---- one gather of KR fp32 ext rows per token ----
                    gth = tp.tile([128, KR, CE], F32, tag="gth")
                    nc.gpsimd.dma_gather(
                        out_ap=gth, in_ap=d["cen_ext"], idxs_ap=idxw,
                        num_idxs=KR * 128, num_idxs_reg=KR * 128, elem_size=CE)
                    # ---- exact fp32 rescue: gpsimd mul + DVE reduce ----
                    sex = sp.tile([NT, KR], F32, tag="sex")
                    for k in range(KR):
                        prod = tp.tile([NT, C + 1], F32, tag="prod")
                        nc.gpsimd.tensor_mul(prod, x_t, gth[:, k, :C + 1])
                        nc.vector.tensor_reduce(sex[:, k:k + 1], prod, AX.X,
                                                ALU.add)
                    # ---- running argmax over exact scores -> winner INDEX ----
                    bv = sp.tile([NT, 1], F32, tag="bv")
                    nc.vector.tensor_copy(out=bv, in_=sex[:, 0:1])
                    wid = sp.tile([NT, 1], I16, tag="wid")
                    nc.vector.tensor_copy(out=wid, in_=idx16c[:, 0:1])
                    for k in range(1, KR):
                        mask = sp.tile([NT, 1], U8, tag=f"mask{k}")
                        nc.vector.tensor_tensor(mask, sex[:, k:k + 1], bv,
                                                ALU.is_gt)
                        nc.vector.copy_predicated(bv, mask, sex[:, k:k + 1])
                        nc.vector.copy_predicated(wid, mask, idx16c[:, k:k + 1])
                    # ---- winner row -> channel-major via transposed gather ----
                    scr2 = dr.tile([16, 8], I16, tag="scr2")
                    nc.scalar.dma_start(out=scr2.rearrange("q j -> j q"), in_=wid)
                    idxw2 = sp.tile([128, 8], I16, tag="idxw2")
                    nc.scalar.dma_start(
                        out=idxw2,
                        in_=scr2.unsqueeze(0).broadcast_to([8, 16, 8]))
                    nc.gpsimd.dma_gather(
                        out_ap=linT16[:, t], in_ap=d["cenT16r"], idxs_ap=idxw2,
                        num_idxs=128, num_idxs_reg=128, elem_size=C,
                        transpose=True)
                    # ---- speaker residual (fp16, DVE) ----
                    nc.vector.tensor_tensor(spk16[:, :, tsl], xhb[:, :, tsl],
                                            linT16[:, t], ALU.subtract)

                pending.append((spk16, linT16, out_b, tb0))
                if len(pending) > 2:
                    emit_mlp(pending.pop(0))
        for prev in pending:
            emit_mlp(prev)


def build_nc():
    nc = bacc.Bacc("TRN2", target_bir_lowering=False, debug=False,
                   enable_asserts=False, num_devices=NCORES)
    d = {}
    d["xh"] = nc.dram_tensor("xh", (BPC, C, TP), F16, kind="ExternalInput").ap()
    d["xT"] = nc.dram_tensor("xT", (BPC, TP, CXT), F32, kind="ExternalInput").ap()
    d["cT16"] = nc.dram_tensor("cT16", (C, NBINS), F16, kind="ExternalInput").ap()
    d["bias2"] = nc.dram_tensor("bias2", (2, NBINS), F16, kind="ExternalInput").ap()
    d["cen_ext"] = nc.dram_tensor("cen_ext", (NBINS, CE), F32,
                                  kind="ExternalInput").ap()
    d["cenT16r"] = nc.dram_tensor("cenT16r", (NBINS, C), F16,
                                  kind="ExternalInput").ap()
    for nm, shp in [("w0T", (C, HID)), ("w1T", (HID, HID)), ("w2T", (HID, EMB)),
                    ("v0T", (EMB, HID)), ("v1T", (HID, HID)), ("v2T", (HID, C))]:
        d[nm] = nc.dram_tensor(nm, shp, F16, kind="ExternalInput").ap()
    for nm, n in [("b0", HID), ("b1", HID), ("b2", EMB),
                  ("c0", HID), ("c1", HID)]:
        d[nm] = nc.dram_tensor(nm, (n,), F32, kind="ExternalInput").ap()
    d["out"] = nc.dram_tensor("out", (BPC, C, TP), F32, kind="ExternalOutput").ap()

    with tile.TileContext(nc) as tc:
        _build_body(nc, tc, d)
    nc.compile()
    return nc


def _prep_shared(centroid, ew0, eb0, ew1, eb1, ew2, eb2, dw0, db0, dw1, db1,
                 dw2, db2):
    cen = np.asarray(centroid, np.float32)
    c_norm = (cen.astype(np.float64) ** 2).sum(1)
    bias32 = (-0.5 * c_norm).astype(np.float32)
    bias_c = bias32 - np.float32(bias32.mean())
    b_hi = bias_c.astype(np.float16)
    b_lo = (bias_c - b_hi.astype(np.float32)).astype(np.float16)
    # db2 folded in: constant-per-token shift of the rescue score (ranking
    # unchanged) and the gathered row becomes lin_dec + db2 directly.
    cen_db2 = cen + np.asarray(db2, np.float32)[None, :]
    cen_ext = np.zeros((NBINS, CE), np.float32)
    cen_ext[:, :C] = cen_db2
    cen_ext[:, C] = bias32
    cen_ext[:, C + 1] = 1.0
    shared = {
        "cT16": np.ascontiguousarray(cen.T).astype(np.float16),
        "bias2": np.stack([b_hi, b_lo]),
        "cen_ext": cen_ext,
        "cenT16r": cen_db2.astype(np.float16),
        "w0T": np.ascontiguousarray(np.asarray(ew0).T).astype(np.float16),
        "w1T": np.ascontiguousarray(np.asarray(ew1).T).astype(np.float16),
        "w2T": np.ascontiguousarray(np.asarray(ew2).T).astype(np.float16),
        "v0T": np.ascontiguousarray(np.asarray(dw0).T).astype(np.float16),
        "v1T": np.ascontiguousarray(np.asarray(dw1).T).astype(np.float16),
        "v2T": np.ascontiguousarray(np.asarray(dw2).T).astype(np.float16),
        "b0": np.asarray(eb0, np.float32), "b1": np.asarray(eb1, np.float32),
        "b2": np.asarray(eb2, np.float32), "c0": np.asarray(db0, np.float32),
        "c1": np.asarray(db1, np.float32),
    }
    return shared


def _get_nc():
    if "nc" not in _CACHE:
        _CACHE["nc"] = build_nc()
    return _CACHE["nc"]


def run(inputs, trace=False):
    feature = np.ascontiguousarray(np.asarray(inputs["feature"], np.float32))
    shared = _prep_shared(**{k: v for k, v in inputs.items() if k != "feature"})
    xh_full = np.zeros((B, C, TP), np.float16)
    xh_full[:, :, :T] = feature.astype(np.float16)
    xT_full = np.zeros((B, TP, CXT), np.float32)
    xT_full[:, :T, :C] = feature.transpose(0, 2, 1)
    xT_full[:, :, C] = 1.0
    nc = _get_nc()
    in_maps = []
    for c in range(NCORES):
        m = dict(shared)
        m["xh"] = np.ascontiguousarray(xh_full[c * BPC:(c + 1) * BPC])
        m["xT"] = np.ascontiguousarray(xT_full[c * BPC:(c + 1) * BPC])
        in_maps.append(m)
    kw = {}
    if trace:
        kw = dict(trace=True, trace_cores=list(range(NCORES)))
    res = bass_utils.run_bass_kernel_spmd(nc, in_maps, core_ids=list(range(NCORES)),
                                          **kw)
    out = np.empty((B, C, T), np.float32)
    for c in range(NCORES):
        out[c * BPC:(c + 1) * BPC] = res.results[c]["out"][:, :, :T]
    return out, res


def kernel(**inputs) -> np.ndarray:
    out, _ = run(inputs, trace=False)
    return out
